# revision 1
# baseline (speedup 1.0000x reference)
import numpy as np
import jax
import jax.numpy as jnp
from jax.sharding import Mesh, PartitionSpec as P
from jax.experimental.shard_map import shard_map
from functools import partial

B = 4096
MAX_N = 64
HID = 128
N_CORES = 8


def _forward(v_types, v_paths, adj, v_sizes, type_embed, path_embed,
             hid_w, hid_b, eps, gin_w1, gin_b1, gin_w2, gin_b2,
             size_w1, size_b1, size_w2, size_b2,
             pool_w1, pool_b1, pool_w2, pool_b2, gp_w, gp_b):
    # identical math to the reference, on a per-shard batch
    feats = jnp.concatenate([type_embed[v_types], path_embed[v_paths]], axis=-1)
    h0 = feats @ hid_w + hid_b  # [b,N,H]
    eps1 = 1.0 + eps[0]
    b = h0.shape[0]

    def step(Hc, xs):
        v, adj_v, hv = xs
        nsum = jnp.einsum('bn,bnh->bh', adj_v, Hc)
        x = eps1 * hv + nsum
        hn = jax.nn.relu(x @ gin_w1 + gin_b1) @ gin_w2 + gin_b2
        Hc = Hc.at[:, v, :].set(hn)
        return Hc, None

    H_init = jnp.zeros((b, MAX_N, HID), h0.dtype)
    xs = (jnp.arange(MAX_N),
          jnp.moveaxis(adj, 1, 0),
          jnp.moveaxis(h0, 1, 0))
    H_final, _ = jax.lax.scan(step, H_init, xs)

    Hf = H_final.reshape(b, MAX_N * HID)
    g = jax.nn.relu(Hf @ pool_w1 + pool_b1) @ pool_w2 + pool_b2
    s = jax.nn.relu(v_sizes @ size_w1 + size_b1) @ size_w2 + size_b2
    out = jnp.concatenate([g, s], axis=-1) @ gp_w + gp_b
    return out


_BATCH_KEYS = ("v_types", "v_paths", "adj", "v_sizes")
_jitted = None


_ARG_NAMES = None


def _build():
    global _jitted, _ARG_NAMES
    if _jitted is not None:
        return _ARG_NAMES
    devs = jax.devices()[:N_CORES]
    mesh = Mesh(np.array(devs), ("x",))

    arg_names = ["v_types", "v_paths", "adj", "v_sizes", "type_embed",
                 "path_embed", "hid_w", "hid_b", "eps", "gin_w1", "gin_b1",
                 "gin_w2", "gin_b2", "size_w1", "size_b1", "size_w2",
                 "size_b2", "pool_w1", "pool_b1", "pool_w2", "pool_b2",
                 "gp_w", "gp_b"]
    in_specs = tuple(P("x") if n in _BATCH_KEYS else P() for n in arg_names)

    fn = shard_map(_forward, mesh=mesh, in_specs=in_specs, out_specs=P("x"),
                   check_rep=False)
    _jitted = jax.jit(fn)
    _ARG_NAMES = arg_names
    return arg_names


def kernel(**inputs) -> np.ndarray:
    arg_names = _build()
    args = []
    for n in arg_names:
        a = np.asarray(inputs[n])
        if a.dtype == np.int64:
            a = a.astype(np.int32)
        elif a.dtype == np.float64:
            a = a.astype(np.float32)
        args.append(a)
    out = _jitted(*args)
    return np.asarray(jax.device_get(out)).astype(np.float32)

